# revision 51
# baseline (speedup 1.0000x reference)
"""Trainium2 Bass kernel: Tacotron-style location-sensitive attention step.

Redesign vs v0 (297us):
  - ALL large tensors host-staged in bf16, pre-transposed/pre-arranged into
    the exact SBUF layouts the matmuls need -> zero on-device transposes,
    half the HBM traffic (25.6 MB/core vs 47 MB f32).
  - Big streams issued from t=0 on the two HWDGE queues (sync + scalar) in
    priority order: LSTM weights -> activations -> proc -> conv windows ->
    enc pairs.  No SWDGE for bulk data.
  - LSTM stays H-sharded (1/8 weights per core).  Each core computes a
    partial query qp = (W_q[:,hj] @ h[hj,:]) for the FULL batch; partials
    are summed across cores.  Exchange options:
      EXCHANGE="remote": 7x remote_dma_broadcast (SBUF->SBUF, 32KB each)
        -- latency ~ a few us, no ncfw barrier.  Sum of partials is
        permutation-invariant so logical/physical core mapping is irrelevant.
      EXCHANGE="ccl": ncfw AllGather via DRAM (slow but battle-tested).
  - Tail fully per-example pipelined: conv -> scores (qry folded in via a
    K=1 accumulate matmul) -> softmax (transpose-free: exp + accum sums,
    cross-partition sum via tiny matmul, 1/sum folded into the weights;
    max-subtraction skipped since |scores| <= sum|W_out| ~ 5) -> context
    matmul streamed against the enc tiles as they land.

kernel(**inputs) takes FULL numpy inputs and returns the FULL [128, 512]
float32 context.
"""

import sys

sys.path.insert(0, "/opt/trn_rl_repo")

import numpy as np
import ml_dtypes

import concourse.bass as bass
import concourse.mybir as mybir
from concourse import bacc
from concourse.bass_utils import run_bass_kernel_spmd
from concourse.bass import _add_dep_helper
from concourse.tile import TileContext

F32 = mybir.dt.float32
BF16 = mybir.dt.bfloat16
AF = mybir.ActivationFunctionType
NPBF = ml_dtypes.bfloat16

B, S, E, P, H, A, F, KW = 128, 1024, 512, 256, 1024, 128, 32, 31
NCORES = 8
BL = B // NCORES        # 16 examples per core
HL = H // NCORES        # 128 h rows per core
PE_DIM = P + E          # 768
NKI = PE_DIM // 128     # 6
NKH = H // 128          # 8
NK = NKI + NKH          # 14
NC_S = S // 128         # 8 s-chunks
TAPS = 2 * KW           # 62
ENC_BUFS = 5            # enc pair tiles in flight (2 examples / 2 MB each)
CONV_BUFS = 8
WIN_CH = 4              # examples per conv-window DMA chunk

EXCHANGE = "remote"        # "remote" | "ccl"
DEBUG_TAPS = False      # extra DRAM outputs for stage-by-stage debugging


def build():
    nc = bacc.Bacc("TRN2", target_bir_lowering=False, debug=False,
                   num_devices=NCORES)

    dp = nc.declare_dram_parameter
    wt_d = dp("wt", [128, 4, NK, HL], BF16, isOutput=False)
    bias_d = dp("bias", [HL, 4], F32, isOutput=False)
    xt_d = dp("xt", [128, NKI, B], BF16, isOutput=False)
    aht_d = dp("aht", [128, NKH, B], BF16, isOutput=False)
    act_d = dp("act", [HL, B], BF16, isOutput=False)
    wqt_d = dp("wqt", [HL, A], BF16, isOutput=False)
    sel_d = dp("sel", [B, BL], BF16, isOutput=False)
    cst_d = dp("cst", [1, A], BF16, isOutput=False)
    wlt_d = dp("wlt", [F, A], BF16, isOutput=False)
    w2_d = dp("w2", [TAPS, F], BF16, isOutput=False)
    wo_d = dp("wo", [1, A], BF16, isOutput=False)
    win_d = dp("win", [TAPS, BL, S], BF16, isOutput=False)
    proc_d = dp("proc", [128, BL, NC_S, A], BF16, isOutput=False)
    enc_d = dp("enc", [128, BL, NC_S, E], BF16, isOutput=False)
    out_d = dp("out", [2, (BL // 2) * E], F32, isOutput=True)
    if DEBUG_TAPS:
        dbg_qsum_d = dp("dbg_qsum", [B, A], F32, isOutput=True)
        dbg_qrow_d = dp("dbg_qrow", [1, BL * A], F32, isOutput=True)
        dbg_conv_d = dp("dbg_conv", [F, S], F32, isOutput=True)
        dbg_sc_d = dp("dbg_sc", [128, BL * NC_S], F32, isOutput=True)
        dbg_wtt_d = dp("dbg_wtt", [128, BL * NC_S], F32, isOutput=True)
        dbg_ps_d = dp("dbg_ps", [128, BL], F32, isOutput=True)
        dbg_rsc_d = dp("dbg_rsc", [128, BL], F32, isOutput=True)

    with TileContext(nc) as tc:
        with (
            tc.tile_pool(name="const", bufs=1) as cpool,
            tc.tile_pool(name="win", bufs=2) as xpool,
            tc.tile_pool(name="conv", bufs=CONV_BUFS) as convpool,
            tc.tile_pool(name="vsb", bufs=4) as vpool,
            tc.tile_pool(name="enc", bufs=ENC_BUFS) as epool,
            tc.tile_pool(name="psA", bufs=1, space="PSUM") as psA,
            tc.tile_pool(name="psV", bufs=2, space="PSUM") as psV,
            tc.tile_pool(name="psX", bufs=2, space="PSUM") as psX,
            tc.tile_pool(name="dram", bufs=1, space="DRAM") as dpool,
        ):
            def mmA(rows, cols):
                t = psA.tile([128, 512], F32, tag="mm")
                return t[:rows, :cols]

            def mmX(rows, cols):
                t = psX.tile([128, 512], F32, tag="x")
                return t[:rows, :cols]

            # ---------------- priority DMAs (HWDGE, both queues) ----------
            # sync queue: wT half, LSTM activations, proc half, win, enc evens
            # scalar queue: wT half, small consts, proc half, win, enc odds
            wT = cpool.tile([128, 4, NK, HL], BF16)
            nc.sync.dma_start(wT[:, 0:2], wt_d[:, 0:2])
            nc.scalar.dma_start(wT[:, 2:4], wt_d[:, 2:4])

            xT = cpool.tile([128, NKI, B], BF16)
            nc.sync.dma_start(xT[:], xt_d[:])
            ahT = cpool.tile([128, NKH, B], BF16)
            nc.sync.dma_start(ahT[:], aht_d[:])
            acT = cpool.tile([HL, B], BF16)
            nc.sync.dma_start(acT[:], act_d[:])
            bias_sb = cpool.tile([HL, 4], F32)
            nc.sync.dma_start(bias_sb[:], bias_d[:])
            wqT = cpool.tile([HL, A], BF16)
            nc.sync.dma_start(wqT[:], wqt_d[:])
            sel_sb = cpool.tile([B, BL], BF16)
            nc.sync.dma_start(sel_sb[:], sel_d[:])

            cst_sb = cpool.tile([1, A], BF16)
            nc.scalar.dma_start(cst_sb[:], cst_d[:])
            wlocT = cpool.tile([F, A], BF16)
            nc.scalar.dma_start(wlocT[:], wlt_d[:])
            w2 = cpool.tile([TAPS, F], BF16)
            nc.scalar.dma_start(w2[:], w2_d[:])
            wo_row = cpool.tile([1, A], BF16)
            nc.scalar.dma_start(wo_row[:], wo_d[:])

            proc_sb = cpool.tile([128, BL, NC_S, A], BF16)
            nc.sync.dma_start(proc_sb[:, 0:8], proc_d[:, 0:8])
            nc.scalar.dma_start(proc_sb[:, 8:16], proc_d[:, 8:16])

            win_tiles = []
            for i in range(BL // WIN_CH):
                wt_t = xpool.tile([TAPS, WIN_CH, S], BF16, tag="win")
                eng = nc.sync if i % 2 == 0 else nc.scalar
                eng.dma_start(wt_t[:], win_d[:, i * WIN_CH:(i + 1) * WIN_CH])
                win_tiles.append(wt_t)

            enc_tiles = []
            for i in range(BL // 2):
                et = epool.tile([128, 2, NC_S, E], BF16, tag="enc")
                eng = nc.sync if i % 2 == 0 else nc.scalar
                eng.dma_start(et[:], enc_d[:, 2 * i:2 * i + 2])
                enc_tiles.append(et)

            # ---------------- constants ----------------------------------
            ones_row_f = cpool.tile([1, 128], F32)
            nc.vector.memset(ones_row_f[:], 1.0)
            ones_col_f = cpool.tile([128, 1], F32)
            nc.vector.memset(ones_col_f[:], 1.0)
            ones_row_b = cpool.tile([1, 128], BF16)
            nc.vector.memset(ones_row_b[:], 1.0)

            # ---------------- LSTM gates (H-shard, full batch) ------------
            # Emitted first so qp / the cross-core sends leave as early as
            # possible; everything below overlaps the wait for peers.
            gate_sb = []
            for g in range(4):
                ps = mmA(HL, B)
                for k in range(NK):
                    rhs = xT[:, k, :] if k < NKI else ahT[:, k - NKI, :]
                    nc.tensor.matmul(ps, wT[:, g, k, :], rhs,
                                     start=(k == 0), stop=(k == NK - 1))
                sb = cpool.tile([HL, B], BF16, tag=f"gate{g}")
                fn = AF.Tanh if g == 2 else AF.Sigmoid
                nc.scalar.activation(sb[:], ps, fn, bias=bias_sb[:, g:g + 1])
                gate_sb.append(sb)

            cT = cpool.tile([HL, B], BF16)
            nc.vector.tensor_mul(cT[:], gate_sb[1][:], acT[:])
            tg = cpool.tile([HL, B], BF16)
            nc.vector.tensor_mul(tg[:], gate_sb[0][:], gate_sb[2][:])
            nc.vector.tensor_add(cT[:], cT[:], tg[:])
            nc.scalar.activation(tg[:], cT[:], AF.Tanh)
            hT = cpool.tile([HL, B], BF16)
            nc.vector.tensor_mul(hT[:], gate_sb[3][:], tg[:])

            # partial query for the FULL batch: qp[b, a]
            ps_q = mmA(B, A)
            nc.tensor.matmul(ps_q, hT[:], wqT[:], start=True, stop=True)
            qp_sb = cpool.tile([B, A], BF16)
            nc.vector.tensor_copy(qp_sb[:], ps_q)

            # ---------------- cross-core exchange: send side --------------
            gather = cpool.tile([B, NCORES, A], BF16)
            qsum = cpool.tile([B, A], BF16)
            deferred_wait = None   # (instruction, sem, value) set post-exit
            if EXCHANGE == "remote":
                qsem = nc.alloc_semaphore("qx_remote")
                lsem = nc.alloc_semaphore("qx_local")
                for k in range(1, NCORES):
                    rd = [None] * NCORES
                    rd[k] = (0, k)
                    nc.gpsimd.remote_dma_broadcast(
                        gather[:, k, :], qp_sb[:],
                        remote_sem=qsem, local_sem=lsem, rdests=rd)
                trig = nc.gpsimd.trigger_dma(count=None)

            # ---------------- qry-independent phase 1 ---------------------
            # wo replicated across partitions and s-chunks
            ps = mmA(128, A)
            nc.tensor.matmul(ps, ones_row_b[:], wo_row[:], start=True,
                             stop=True)
            wo_rep8 = cpool.tile([128, NC_S, A], BF16)
            for c in range(NC_S):
                if c % 2:
                    nc.scalar.copy(wo_rep8[:, c, :], ps)
                else:
                    nc.vector.tensor_copy(wo_rep8[:, c, :], ps)

            # conv + location scores, merged into proc in place:
            #   proc_sb[:, b] <- conv(b)^T . Wloc^T + proc_sb[:, b]
            for b in range(BL):
                wt_t = win_tiles[b // WIN_CH]
                bi = b % WIN_CH
                conv_sb = convpool.tile([F, S], BF16, tag="conv")
                for h2 in range(2):
                    ps_c = mmA(F, 512)
                    nc.tensor.matmul(ps_c, w2[:],
                                     wt_t[:, bi, h2 * 512:(h2 + 1) * 512],
                                     start=True, stop=True)
                    if h2 == 0:
                        nc.vector.tensor_copy(
                            conv_sb[:, h2 * 512:(h2 + 1) * 512], ps_c)
                    else:
                        p1_act = nc.scalar.copy(
                            conv_sb[:, h2 * 512:(h2 + 1) * 512], ps_c)
                if DEBUG_TAPS and b == 0:
                    t = cpool.tile([F, S], F32, tag="dbgc")
                    nc.vector.tensor_copy(t[:], conv_sb[:F, :])
                    nc.sync.dma_start(dbg_conv_d[:], t[:])
                ps_v = psV.tile([128, NC_S * A], F32, tag="v", bufs=1)
                for c in range(NC_S):
                    p1_mm = nc.tensor.matmul(ps_v[:, c * A:(c + 1) * A],
                                             conv_sb[:, c:S:NC_S], wlocT[:],
                                             start=True, stop=True)
                p1_dve = nc.vector.tensor_add(
                    proc_sb[:, b],
                    ps_v[:].rearrange("p (c a) -> p c a", c=NC_S),
                    proc_sb[:, b])

            # ---------------- exchange: receive side ----------------------
            if EXCHANGE == "remote":
                add0 = nc.gpsimd.tensor_add(qsum[:], qp_sb[:],
                                            gather[:, 1, :])
                _add_dep_helper(add0.ins, trig.ins, sync=True,
                                reason="consume gather only after trigger")
                deferred_wait = (add0, qsem, 2 * (NCORES - 1))
                for k in range(2, NCORES):
                    nc.gpsimd.tensor_add(qsum[:], qsum[:],
                                         gather[:, k, :])
            else:
                qp_dram = dpool.tile([B, A], BF16)
                nc.sync.dma_start(qp_dram[:], qp_sb[:])
                gat_dram = dpool.tile([NCORES, B, A], BF16)
                nc.gpsimd.collective_compute(
                    "AllGather",
                    mybir.AluOpType.bypass,
                    replica_groups=[list(range(NCORES))],
                    ins=[qp_dram[:].opt()],
                    outs=[gat_dram[:].opt()],
                )
                nc.gpsimd.dma_start(
                    gather[:], gat_dram[:].rearrange("c b a -> b c a"))
                nc.vector.tensor_add(qsum[:], gather[:, 0, :],
                                     gather[:, 1, :])
                for k in range(2, NCORES):
                    nc.vector.tensor_add(qsum[:], qsum[:], gather[:, k, :])

            if DEBUG_TAPS:
                t = cpool.tile([B, A], F32, tag="dbgq")
                nc.vector.tensor_copy(t[:], qsum[:])
                nc.sync.dma_start(dbg_qsum_d[:], t[:])

            # select this core's 16 examples, add folded consts
            ps_s = mmA(BL, A)
            sel_mm = nc.tensor.matmul(ps_s, sel_sb[:], qsum[:],
                                      start=True, stop=False)
            # The scheduler cannot see the deferred remote-sem wait, so pin
            # every engine's phase-1 work BEFORE the first post-wait op --
            # otherwise the engine streams stall at the wait with phase-1
            # still queued behind it.
            _add_dep_helper(sel_mm.ins, p1_mm.ins, sync=False,
                            reason="phase-1 PE before post-wait PE")
            nc.tensor.matmul(ps_s, ones_row_b[:, :BL], cst_sb[:],
                             start=False, stop=True)
            qry2 = cpool.tile([BL, A], BF16)
            q2cp = nc.vector.tensor_copy(qry2[:], ps_s)
            _add_dep_helper(q2cp.ins, p1_dve.ins, sync=False,
                            reason="phase-1 DVE before post-wait DVE")
            # move the 16 query rows onto partition 0 (free dim) with a tiny
            # SBUF->SBUF DMA on the gpsimd queue (PE operands must sit at
            # base partition 0/32/64).
            qrow = cpool.tile([1, BL, A], BF16)
            nc.gpsimd.dma_start(qrow[:], qry2[:])
            if DEBUG_TAPS:
                t = cpool.tile([1, BL * A], F32, tag="dbgqr")
                nc.vector.tensor_copy(
                    t[:], qrow[:].rearrange("p b a -> p (b a)"))
                nc.sync.dma_start(dbg_qrow_d[:], t[:])
            # Fire-and-forget tiny collective, last on gpsimd: its presence
            # makes the runtime set up the global comm, which aligns NEFF
            # starts across cores to ~tens of us (vs ~10ms+ without).  Its
            # latency hides under the compute tail; nothing consumes it.
            if EXCHANGE == "remote":
                align_in = dpool.tile([1, 128], F32)
                align_out = dpool.tile([1, 128], F32)
                nc.gpsimd.collective_compute(
                    "AllReduce",
                    mybir.AluOpType.add,
                    replica_groups=[list(range(NCORES))],
                    ins=[align_in[:].opt()],
                    outs=[align_out[:].opt()],
                )

            # ---------------- post-qry phase 2 ----------------------------
            sc_f = cpool.tile([128, BL, NC_S], F32)     # scores
            wTt = cpool.tile([128, BL, NC_S], BF16)     # softmax weights
            psums = cpool.tile([128, BL], F32)
            rcp = cpool.tile([1, BL], F32)
            rsc = cpool.tile([128, BL], F32)

            out_sb = None
            ps_q4 = None
            for b in range(BL):
                # qry for 4 examples replicated across partitions in ONE
                # N=512 matmul; v = tanh(u + qry) . wo, reduced over a
                if b % 4 == 0:
                    ps_q4 = psV.tile([128, 4 * A], F32, tag="q4", bufs=1)
                    nc.tensor.matmul(ps_q4, ones_row_b[:, :128],
                                     qrow[0:1, b:b + 4, :].rearrange(
                                         "p b a -> p (b a)"),
                                     start=True, stop=True)
                    # drain to bf16 SBUF right away: frees the PSUM bank for
                    # the next group and lets the v-add run at 2x DVE rate
                    qrep_sb = vpool.tile([128, 4, A], BF16, tag="qr",
                                         bufs=2)
                    nc.scalar.copy(
                        qrep_sb[:].rearrange("p b a -> p (b a)"), ps_q4)
                if b % 2 == 1:
                    continue        # v-chain handled in pairs at even b
                qslc = qrep_sb[:, b % 4:b % 4 + 2, :]
                qrep_bcast = bass.AP(
                    tensor=qslc.tensor, offset=qslc.offset,
                    ap=[list(qslc.ap[0]), [A, 2], [0, NC_S], [1, A]])
                wo_bcast = bass.AP(
                    tensor=wo_rep8[:].tensor, offset=wo_rep8[:].offset,
                    ap=[list(wo_rep8[:].ap[0]), [0, 2], [A, NC_S], [1, A]])
                v_sb = vpool.tile([128, 2, NC_S, A], BF16, tag="v_sb",
                                  bufs=2)
                v_add = nc.vector.tensor_add(v_sb[:], qrep_bcast,
                                             proc_sb[:, b:b + 2])
                if b == 0:
                    _add_dep_helper(v_add.ins, p1_dve.ins, sync=False,
                                    reason="phase-1 DVE before post-wait DVE")
                th = nc.scalar.activation(v_sb[:], v_sb[:], AF.Tanh)
                if b == 0:
                    _add_dep_helper(th.ins, p1_act.ins, sync=False,
                                    reason="phase-1 ACT before post-wait ACT")
                nc.vector.tensor_mul(v_sb[:], v_sb[:], wo_bcast)
                nc.vector.reduce_sum(sc_f[:, b:b + 2, :], v_sb[:],
                                     axis=mybir.AxisListType.X)

                # softmax exp (no max-subtraction); per-partition sums
                for be in (b, b + 1):
                    nc.scalar.activation(wTt[:, be, :], sc_f[:, be, :],
                                         AF.Exp,
                                         accum_out=psums[:, be:be + 1])
                b = b + 1   # group boundary check below uses the pair end

                if b % 4 != 3:
                    continue
                # batched softmax normalization for this group of 4:
                # cross-partition total, reciprocal, replicate, scale.
                g0 = b - 3
                ps_m = mmA(1, 4)
                nc.tensor.matmul(ps_m, ones_col_f[:], psums[:, g0:b + 1],
                                 start=True, stop=True)
                nc.vector.reciprocal(rcp[:, g0:b + 1], ps_m)
                for bb in range(g0, b + 1):
                    if bb % 8 == 0:
                        out_sb = cpool.tile([1, 8, E], F32, tag="out")
                    # context with UNNORMALIZED exp weights; 1/Z folded into
                    # the single-row PSUM drain below.
                    et = enc_tiles[bb // 2]
                    ps_xt = psX.tile([128, 512], F32, name="ps_xt",
                                     tag=("x" if bb % 2 else "x2"))
                    ps_x = ps_xt[:1, :E]
                    for c in range(NC_S):
                        nc.tensor.matmul(ps_x, wTt[:, bb, c:c + 1],
                                         et[:, bb % 2, c, :],
                                         start=(c == 0), stop=(c == NC_S - 1))
                    nc.vector.tensor_scalar_mul(out_sb[:, bb % 8, :], ps_x,
                                                rcp[:, bb:bb + 1])
                    if bb % 8 == 7:
                        nc.sync.dma_start(
                            out_d[bb // 8:bb // 8 + 1, :],
                            out_sb[:].rearrange("p b e -> p (b e)"))
            if DEBUG_TAPS:
                nc.sync.dma_start(
                    dbg_sc_d[:], sc_f[:].rearrange("p b c -> p (b c)"))
                tw = cpool.tile([128, BL, NC_S], F32, tag="dbgw")
                nc.vector.tensor_copy(tw[:], wTt[:])
                nc.sync.dma_start(
                    dbg_wtt_d[:], tw[:].rearrange("p b c -> p (b c)"))
                nc.sync.dma_start(dbg_ps_d[:], psums[:])
                nc.sync.dma_start(dbg_rsc_d[:], rsc[:])

    if deferred_wait is not None:
        # Post-scheduling: bake the remote-sem wait into the first consumer
        # of the gathered slots.  The scheduler never simulates it (it would
        # deadlock -- the increments come from peer cores); the NEFF gets it.
        ins, sem, val = deferred_wait
        ins.wait_op(sem, val, "sem-ge", check=False)

    nc.compile()
    return nc


_NC_CACHE = None


def _get_nc():
    global _NC_CACHE
    if _NC_CACHE is None:
        _NC_CACHE = build()
    return _NC_CACHE


def shard_inputs(prenet, prev_context, att_h, att_c, prev_weights, cum_weights,
                 enc_seq, proc_mem, mask, W_ih, W_hh, b_ih, b_hh, conv_w,
                 conv_b, W_loc, b_loc, W_q, b_q, W_out, **_unused):
    f32 = np.float32
    c = np.ascontiguousarray

    def bf(x):
        return c(np.asarray(x).astype(NPBF))

    W4 = np.concatenate([np.asarray(W_ih, f32).reshape(4, H, PE_DIM),
                         np.asarray(W_hh, f32).reshape(4, H, H)], axis=2)
    bias4 = (np.asarray(b_ih, f32) + np.asarray(b_hh, f32)).reshape(4, H)

    x_full = np.concatenate([np.asarray(prenet, f32),
                             np.asarray(prev_context, f32)], axis=1)  # [B,768]
    xt_host = bf(x_full.T.reshape(NKI, 128, B).transpose(1, 0, 2))
    aht_host = bf(np.asarray(att_h, f32).T.reshape(NKH, 128, B)
                  .transpose(1, 0, 2))

    cst_host = bf((np.asarray(W_loc, f32) @ np.asarray(conv_b, f32)
                   + np.asarray(b_loc, f32)
                   + np.asarray(b_q, f32)).reshape(1, A))
    wlt_host = bf(np.asarray(W_loc, f32).T)                     # [F, A]
    w2_host = bf(np.asarray(conv_w, f32).transpose(1, 2, 0).reshape(TAPS, F))
    wo_host = bf(np.asarray(W_out, f32).reshape(1, A))

    cum = np.asarray(cum_weights, f32)
    prev = np.asarray(prev_weights, f32)
    att_c = np.asarray(att_c, f32)
    W_q = np.asarray(W_q, f32)
    enc_seq = np.asarray(enc_seq, f32)
    proc_mem = np.asarray(proc_mem, f32)

    in_maps = []
    for j in range(NCORES):
        bj = slice(BL * j, BL * (j + 1))
        hj = slice(HL * j, HL * (j + 1))

        W4s = W4[:, hj, :]                                      # [4,128,1792]
        wt_host = bf(W4s.reshape(4, HL, NK, 128).transpose(3, 0, 2, 1))

        sel = np.zeros((B, BL), f32)
        sel[bj, :] = np.eye(BL, dtype=f32)

        xp = np.zeros((BL, 2, S + KW - 1), f32)
        xp[:, 0, KW // 2:KW // 2 + S] = cum[bj]
        xp[:, 1, KW // 2:KW // 2 + S] = prev[bj]
        sw = np.lib.stride_tricks.sliding_window_view(xp, S, axis=2)
        win_host = bf(sw.transpose(1, 2, 0, 3).reshape(TAPS, BL, S))

        proc_host = bf(proc_mem[bj].reshape(BL, 128, NC_S, A)
                       .transpose(1, 0, 2, 3))
        enc_host = bf(enc_seq[bj].reshape(BL, 128, NC_S, E)
                      .transpose(1, 0, 2, 3))

        in_maps.append({
            "wt": wt_host,
            "bias": c(bias4[:, hj].T),
            "xt": xt_host,
            "aht": aht_host,
            "act": bf(att_c[:, hj].T),
            "wqt": bf(W_q[:, hj].T),
            "sel": bf(sel),
            "cst": cst_host,
            "wlt": wlt_host,
            "w2": w2_host,
            "wo": wo_host,
            "win": win_host,
            "proc": proc_host,
            "enc": enc_host,
        })
    return in_maps


def kernel(**inputs):
    assert not np.any(np.asarray(inputs["mask"])), \
        "kernel assumes mask == 0 (softmax-shift support not implemented)"
    nc = _get_nc()
    in_maps = shard_inputs(**inputs)
    res = run_bass_kernel_spmd(nc, in_maps, core_ids=list(range(NCORES)))
    return np.concatenate(
        [np.asarray(res.results[j]["out"]).reshape(BL, E)
         for j in range(NCORES)], axis=0)


if __name__ == "__main__":
    print("building...")
    _get_nc()
    print("built ok")
